# revision 2
# baseline (speedup 1.0000x reference)
"""Trainium2 Bass kernel for nn_MetricLoss — v2 (label-sorted, fp8 DoubleRow).

Reference (N=4096, F=512, 16 classes):
    Dsq = ||b_i||^2 + ||a_j||^2 - 2 b@a.T ;  D = sqrt(Dsq)        [N,N]
    Dexpm = exp(1 - D);  ns[i] = sum_{lbl_j != lbl_i} Dexpm[i,j]
    J = ln(ns_i + ns_j) + D;  loss = sum_{i!=j, same lbl} relu(J)^2/(2 num_pos)

v2 design (vs the v1 transposed-layout kernel):
  * Host sorts rows by label; positives collapse into a narrow block-diagonal
    band (W columns per core) -> phase 2 (ln + hinge) shrinks ~16x.
  * NORMAL layout (i on partitions): row sums ride the Exp activation's
    accumulator (no more one-hot reduction matmuls), ns = total - samesum
    with samesum a one-DVE masked sum over the band.
  * fp8 e4m3 DoubleRow GEMM (2 matmuls per 512-K psum tile instead of 4);
    ||b||^2 rides the Sqrt activation bias (fp32 exact), ||a||^2 rides a
    K=2 bf16 hi/lo augmented matmul.
  * Per-core 128-aligned column ROTATION (host-side) makes every band slice
    core-uniform so one SPMD program serves all 8 cores; the gathered ns is
    re-indexed on-device via a host-built one-hot selector (eqx) + ones
    matmul instead of a core-dependent DMA offset.
  * The diagonal (i==j) is excluded by the hinge mask directly; no separate
    diagonal correction path at all.
"""

import re
import operator
import numpy as np
import ml_dtypes
from contextlib import ExitStack

import concourse.bass as bass
import concourse.tile as tile
from concourse import bacc, mybir
from concourse import dve_ops
from concourse.dve_spec import Spec, Src0, Src1, C0, relu, sq
from concourse.bass_utils import run_bass_kernel_spmd
from concourse.tile_rust import add_dep_helper

F32 = mybir.dt.float32
BF16 = mybir.dt.bfloat16
FP8 = mybir.dt.float8e4
NPBF16 = ml_dtypes.bfloat16
NPE4M3 = ml_dtypes.float8_e4m3
AF = mybir.ActivationFunctionType
ALU = mybir.AluOpType
DR = mybir.MatmulPerfMode.DoubleRow

N = 4096          # rows (a and b)
F = 512           # features
NCORES = 8
R = N // NCORES   # rows of b per core = 512
TI = R // 128     # i-tiles per core = 4
NJ = N // 512     # j-chunks of 512 = 8
NCLS = 16


def _register_custom(name, spec):
    for op in dve_ops.OPS:
        if op.name == name:
            return op
    op = dve_ops.DveOp(name, spec, subdim=False, uops_sha={})
    dve_ops._SUB_OPCODE_FOR_NAME[name] = (
        max(dve_ops._SUB_OPCODE_FOR_NAME.values()) + 1)
    assert dve_ops._SUB_OPCODE_FOR_NAME[name] < 0x20
    for ver in ("v3", "v4"):
        try:
            op.compile(ver)
        except ValueError as e:
            m = re.search(r"\(%s: ([0-9a-f]+) " % ver, str(e))
            if not m:
                raise
            op.uops_sha[ver] = m.group(1)
            op.compile(ver)
    dve_ops.OPS.append(op)
    dve_ops.CUSTOM_DVE_SPECS[name] = op.spec
    return op


def _register_sqrelu_add():
    """Custom fused DVE op: out = relu(in0 + in1)^2, accum_out = c0 + sum(out)."""
    def _ref(in0, in1, c0, c1, c2):
        out = np.square(np.maximum(
            in0.astype(np.float32) + in1.astype(np.float32), 0.0))
        acc = c0 + out.reshape(out.shape[0], -1).sum(axis=1, keepdims=True)
        return out, acc

    return _register_custom(
        "SQRELU_ADD_ANT",
        Spec(body=sq(relu(Src0 + Src1)), accum=operator.add, accum_init=C0,
             reference=_ref))


def _register_mul_acc():
    """Custom fused DVE op: out = in0 * in1, accum_out = c0 + sum(out)."""
    def _ref(in0, in1, c0, c1, c2):
        out = in0.astype(np.float32) * in1.astype(np.float32)
        acc = c0 + out.reshape(out.shape[0], -1).sum(axis=1, keepdims=True)
        return out, acc

    return _register_custom(
        "MUL_ACC_ANT",
        Spec(body=Src0 * Src1, accum=operator.add, accum_init=C0,
             reference=_ref))


def build_bass(W: int):
    NW = W // 512     # 512-wide band chunks
    NBQ = W // 128
    sqrelu_add = _register_sqrelu_add()
    mul_acc = _register_mul_acc()

    nc = bacc.Bacc("TRN2", target_bir_lowering=False, debug=False,
                   num_devices=NCORES)

    # ---- kernel I/O (per-core shards prepared on host) ----
    at8 = nc.dram_tensor("at8", [128, 4, N], FP8, kind="ExternalInput").ap()      # a_rot.T  [p,k,j]
    bt8 = nc.dram_tensor("bt8", [128, 4, R], FP8, kind="ExternalInput").ap()      # (-2 b_c).T [p,k,i]
    aabc = nc.dram_tensor("aabc", [128, NJ, 512], F32, kind="ExternalInput").ap() # ||a_j||^2 bcast (rotated)
    bb4 = nc.dram_tensor("bb4", [128, TI], F32, kind="ExternalInput").ap()        # ||b_i||^2
    msame = nc.dram_tensor("msame", [128, TI, NW, 512], BF16, kind="ExternalInput").ap()
    mneg = nc.dram_tensor("mneg", [128, TI, NW, 512], BF16, kind="ExternalInput").ap()
    eqx = nc.dram_tensor("eqx", [32, W], BF16, kind="ExternalInput").ap()         # gather-block select
    eye128 = nc.dram_tensor("eye128", [128, 128], F32, kind="ExternalInput").ap()

    out_same = nc.dram_tensor("out_same", [1, 1], F32, kind="ExternalOutput").ap()
    out_ns = nc.dram_tensor("out_ns", [1, R], F32, kind="ExternalOutput").ap()

    with tile.TileContext(nc) as tc, ExitStack() as ctx:
        sb = ctx.enter_context(tc.tile_pool(name="sb", bufs=1))
        dexp_p = ctx.enter_context(tc.tile_pool(name="dexp", bufs=2))
        lp = ctx.enter_context(tc.tile_pool(name="lp", bufs=2))
        scr = ctx.enter_context(tc.tile_pool(name="scr", bufs=2))
        dram = ctx.enter_context(tc.tile_pool(name="dram", bufs=1, space="DRAM"))

        # ---- collective warmup path first: fire w1 at t~0 ----
        warm_sb = sb.tile([1, 8], F32)
        warm2_sb = sb.tile([1, R], F32)
        cc_in = dram.tile([1, R], F32)
        cc_out = dram.tile([1, N], F32)
        warm_in = dram.tile([1, 8], F32)
        warm_out = dram.tile([1, 8 * NCORES], F32)
        warm2_in = dram.tile([1, R], F32)
        warm2_out = dram.tile([1, N], F32)
        with tc.high_priority():
            nc.vector.memset(warm_sb, 0.0)
            nc.sync.dma_start(out=warm_in, in_=warm_sb)
            w1 = nc.gpsimd.collective_compute(
                "AllGather", ALU.bypass,
                replica_groups=[list(range(NCORES))],
                ins=[warm_in[:].opt()], outs=[warm_out[:].opt()])

        # ---- PE pre-warm: ~6us of dummy matmuls so HAM reaches 8/8 before
        # the real GEMM on EVERY core (cold cores straggle the collective) ----
        wsrc = sb.tile([128, 512], BF16)
        with tc.high_priority():
            nc.vector.memset(wsrc, 0.0)
            with tc.tile_pool(name="warm_ps", bufs=1, space="PSUM") as wpool:
                wps = wpool.tile([128, 512], F32)
                for _ in range(16):
                    nc.tensor.matmul(out=wps, lhsT=wsrc[:, 0:128], rhs=wsrc,
                                     start=True, stop=True)

        # ---- resident SBUF tensors (GEMM-critical first) ----
        bt8_sb = sb.tile([128, 4, R], FP8)
        nc.sync.dma_start(out=bt8_sb, in_=bt8)
        bb_sb = sb.tile([128, TI], F32)
        nc.sync.dma_start(out=bb_sb, in_=bb4)
        at8_sb = sb.tile([128, 4, N], FP8)
        for q in range(4):
            nc.sync.dma_start(out=at8_sb[:, :, 1024 * q:1024 * q + 1024],
                              in_=at8[:, :, 1024 * q:1024 * q + 1024])
        aabc_sb = sb.tile([128, NJ, 512], F32)
        for q in range(4):
            nc.sync.dma_start(out=aabc_sb[:, 2 * q:2 * q + 2, :],
                              in_=aabc[:, 2 * q:2 * q + 2, :])

        ones32 = sb.tile([32, 128], BF16)
        nc.vector.memset(ones32, 1.0)
        ones128c = sb.tile([128, 1], F32)
        nc.vector.memset(ones128c, 1.0)

        dT = sb.tile([128, TI, NJ, 512], F32)     # D, 64KB/partition
        dm_sb = sb.tile([128, TI, NW, 512], F32)  # D + hinge mask over band
        tot_sb = sb.tile([128, TI], F32)
        sam_sb = sb.tile([128, TI], F32)
        ns_sb = sb.tile([128, TI], F32)
        hacc_sb = sb.tile([128, TI], F32)
        nst_sb = sb.tile([TI, 128], F32)
        nsflat_sb = sb.tile([32, 128], F32)
        nsqe_sb = sb.tile([32, W], BF16)

        # later resident loads (not on the GEMM critical path)
        msame_sb = sb.tile([128, TI, NW, 512], BF16)
        nc.sync.dma_start(out=msame_sb, in_=msame)
        mneg_sb = sb.tile([128, TI, NW, 512], BF16)
        nc.sync.dma_start(out=mneg_sb, in_=mneg)
        eqx_sb = sb.tile([32, W], BF16)
        nc.sync.dma_start(out=eqx_sb, in_=eqx)
        eye128_sb = sb.tile([128, 128], F32)
        nc.sync.dma_start(out=eye128_sb, in_=eye128)

        # ========== PHASE 1: GEMM (fp8 DoubleRow) -> +aa (DVE) -> sqrt =====
        sqrt_insts = []
        with tc.tile_pool(name="dsq_ps", bufs=2, space="PSUM") as dsq_pool, \
             tc.tile_pool(name="dsq_sb", bufs=2) as dsq_sbp:
            for t in range(TI):
                lw01 = bt8_sb[:, 0:2, 128 * t:128 * t + 128]
                lw23 = bt8_sb[:, 2:4, 128 * t:128 * t + 128]
                for h in range(2):
                    ps = dsq_pool.tile([128, 4, 512], F32, tag="dsq")
                    for u in range(4):
                        q = 4 * h + u
                        jsl = slice(512 * q, 512 * q + 512)
                        nc.tensor.matmul(out=ps[:, u, :], lhsT=lw01,
                                         rhs=at8_sb[:, 0:2, jsl],
                                         perf_mode=DR, start=True, stop=False)
                        nc.tensor.matmul(out=ps[:, u, :], lhsT=lw23,
                                         rhs=at8_sb[:, 2:4, jsl],
                                         perf_mode=DR, start=False, stop=True)
                    dsq = dsq_sbp.tile([128, 4, 512], F32, tag="dsqs")
                    nc.vector.tensor_add(out=dsq, in0=ps,
                                         in1=aabc_sb[:, 4 * h:4 * h + 4, :])
                    si = nc.scalar.activation(
                        out=dT[:, t, 4 * h:4 * h + 4, :], in_=dsq,
                        func=AF.Sqrt, bias=bb_sb[:, t:t + 1], scale=1.0)
                    sqrt_insts.append(si)

        # ========== exp (+row totals via ACT accumulate), samesum ==========
        prev = sqrt_insts[-1]
        for t in range(TI):
            dexp = dexp_p.tile([128, NJ, 512], BF16, tag="dexp")
            ei = nc.scalar.activation(
                out=dexp, in_=dT[:, t, :, :], func=AF.Exp,
                scale=-1.0, bias=1.0, accum_out=tot_sb[:, t:t + 1])
            add_dep_helper(ei.ins, prev.ins, False, "ACT table order")
            prev = ei
            sscr = scr.tile([128, NW, 512], BF16, tag="sscr")
            nc.vector._custom_dve(mul_acc, out=sscr, in0=dexp[:, 0:NW, :],
                                  in1=msame_sb[:, t, :, :],
                                  s0=0.0, accum_out=sam_sb[:, t:t + 1])
            # hinge-masked D band (runs on DVE during the exp window)
            nc.vector.tensor_add(out=dm_sb[:, t, :, :],
                                 in0=dT[:, t, 0:NW, :],
                                 in1=mneg_sb[:, t, :, :])

        nc.vector.tensor_sub(ns_sb, tot_sb, sam_sb)

        # ========== ns -> row layout -> AllGather ==========
        with tc.tile_pool(name="ps2", bufs=1, space="PSUM") as ps2, \
             tc.tile_pool(name="nsbc_ps", bufs=1, space="PSUM") as nsbc_pool:
            nst_ps = ps2.tile([TI, 128], F32, tag="nst")
            nc.tensor.matmul(out=nst_ps, lhsT=ns_sb, rhs=eye128_sb,
                             start=True, stop=True)
            nc.vector.tensor_copy(out=nst_sb, in_=nst_ps)
            nc.sync.dma_start(out=cc_in[0, :].rearrange("(t p) -> t p", p=128),
                              in_=nst_sb)
            nc.sync.dma_start(out=out_ns[0, :].rearrange("(t p) -> t p", p=128),
                              in_=nst_sb)
            cc_inst = nc.gpsimd.collective_compute(
                "AllGather", ALU.bypass,
                replica_groups=[list(range(NCORES))],
                ins=[cc_in[:].opt()], outs=[cc_out[:].opt()])
            rd = nc.sync.dma_start(
                out=nsflat_sb, in_=cc_out[0, :].rearrange("(q p) -> q p", p=128))
            add_dep_helper(rd.ins, cc_inst.ins, True, "read ns after collective")

            # ns_bc[p, x] = ns[(s_c + x) mod N]  (selector matmul, core-uniform)
            nc.vector.tensor_mul(
                out=nsqe_sb.rearrange("q (y m) -> q y m", m=128),
                in0=nsflat_sb.rearrange("q (o p) -> q o p", o=1)
                    .broadcast_to([32, NBQ, 128]),
                in1=eqx_sb.rearrange("q (y m) -> q y m", m=128))
            nsbc_ps = nsbc_pool.tile([128, NW, 512], F32, tag="nsbc")
            for v in range(NW):
                nc.tensor.matmul(out=nsbc_ps[:, v, :], lhsT=ones32,
                                 rhs=nsqe_sb[:, 512 * v:512 * v + 512],
                                 start=True, stop=True)

            # ========== PHASE 2: J = ln(ns_i + ns_j) + D; hinge^2 ==========
            for t in range(TI):
                L = lp.tile([128, NW, 512], F32, tag="L")
                nc.scalar.activation(out=L, in_=nsbc_ps, func=AF.Ln,
                                     bias=ns_sb[:, t:t + 1], scale=1.0)
                h2 = scr.tile([128, NW, 512], BF16, tag="h2")
                nc.vector._custom_dve(sqrelu_add, out=h2, in0=L,
                                      in1=dm_sb[:, t, :, :],
                                      s0=0.0, accum_out=hacc_sb[:, t:t + 1])

            fin_ps = ps2.tile([1, TI], F32, tag="fin")
            nc.tensor.matmul(out=fin_ps, lhsT=ones128c, rhs=hacc_sb,
                             start=True, stop=True)
            same_sum = sb.tile([1, 1], F32)
            nc.vector.reduce_sum(out=same_sum, in_=fin_ps,
                                 axis=mybir.AxisListType.X)
            nc.sync.dma_start(out=out_same, in_=same_sum)

    nc.compile()
    return nc


_CACHE: dict = {}


def _get_nc(W: int):
    key = ("nc", W)
    if key not in _CACHE:
        _CACHE[key] = build_bass(W)
    return _CACHE[key]


def prepare_inputs(a: np.ndarray, b: np.ndarray, labels: np.ndarray):
    """Host-side sharding/layout prep. Returns (per-core input maps, num_pos, W)."""
    a = np.asarray(a, np.float32)
    b = np.asarray(b, np.float32)
    labels = np.asarray(labels).astype(np.int64)
    perm = np.argsort(labels, kind="stable")
    a_s, b_s, ls = a[perm], b[perm], labels[perm]
    counts = np.bincount(ls, minlength=NCLS)
    starts = np.concatenate([[0], np.cumsum(counts)])
    num_pos = float((counts.astype(np.float64) ** 2).sum() - N)

    bands = []
    for c in range(NCORES):
        l0, l1 = ls[R * c], ls[R * c + R - 1]
        bands.append((int(starts[l0]), int(starts[l1 + 1])))
    maxw = max(j1 - 128 * (j0 // 128) for j0, j1 in bands)
    W = int(np.ceil(maxw / 512) * 512)

    aa = np.sum(a_s * a_s, axis=1, dtype=np.float32)
    bb = np.sum(b_s * b_s, axis=1, dtype=np.float32)
    eye = np.eye(128, dtype=np.float32)

    in_maps = []
    for c in range(NCORES):
        j0, _ = bands[c]
        s = 128 * (j0 // 128)
        cols = (s + np.arange(N)) % N
        a_rot = a_s[cols]
        at8 = np.ascontiguousarray(
            a_rot.T.reshape(4, 128, N).transpose(1, 0, 2)).astype(NPE4M3)
        aar = aa[cols]
        aabc = np.ascontiguousarray(
            np.broadcast_to(aar[None, :], (128, N)).reshape(128, NJ, 512))
        sl = slice(R * c, R * c + R)
        bt8 = np.ascontiguousarray(
            (-2.0 * b_s[sl]).T.reshape(4, 128, R).transpose(1, 0, 2)).astype(NPE4M3)
        bb4 = np.ascontiguousarray(bb[sl].reshape(TI, 128).T)

        collab = ls[cols[:W]]
        rowlab = np.ascontiguousarray(ls[sl].reshape(TI, 128).T)
        same = rowlab[:, :, None] == collab[None, None, :]      # [128, TI, W]
        gidx = (R * c + 128 * np.arange(TI)[None, :]
                + np.arange(128)[:, None])
        isdiag = gidx[:, :, None] == cols[:W][None, None, :]
        msame = same.astype(NPBF16).reshape(128, TI, W // 512, 512)
        mneg = np.where(same & ~isdiag, np.float32(0.0),
                        np.float32(-30000.0)).astype(NPBF16)
        mneg = mneg.reshape(128, TI, W // 512, 512)
        qb0 = s // 128
        eqxm = (np.arange(32)[:, None]
                == ((qb0 + np.arange(W) // 128) % 32)[None, :]).astype(NPBF16)
        in_maps.append({
            "at8": at8, "bt8": bt8, "aabc": aabc, "bb4": bb4,
            "msame": np.ascontiguousarray(msame),
            "mneg": np.ascontiguousarray(mneg),
            "eqx": np.ascontiguousarray(eqxm), "eye128": eye,
        })
    return in_maps, num_pos, W


def run(a, b, labels, trace=False, trace_kwargs=None):
    """Run on 8 NeuronCores; returns (loss, BassKernelResults)."""
    in_maps, num_pos, W = prepare_inputs(a, b, labels)
    nc = _get_nc(W)
    kw = {}
    if trace:
        kw = dict(trace=True, **(trace_kwargs or {}))
    res = run_bass_kernel_spmd(nc, in_maps, core_ids=list(range(NCORES)), **kw)

    total = 0.0
    for c in range(NCORES):
        total += float(res.results[c]["out_same"][0, 0])
    loss = total / (2.0 * num_pos)
    return np.asarray(np.float32(loss)), res


def kernel(a, b, labels):
    loss, _ = run(a, b, labels)
    return loss


# revision 3
# speedup vs baseline: 1.0175x; 1.0175x over previous
"""Trainium2 Bass kernel for nn_MetricLoss — v2 (label-sorted, fp8 DoubleRow).

Reference (N=4096, F=512, 16 classes):
    Dsq = ||b_i||^2 + ||a_j||^2 - 2 b@a.T ;  D = sqrt(Dsq)        [N,N]
    Dexpm = exp(1 - D);  ns[i] = sum_{lbl_j != lbl_i} Dexpm[i,j]
    J = ln(ns_i + ns_j) + D;  loss = sum_{i!=j, same lbl} relu(J)^2/(2 num_pos)

v2 design (vs the v1 transposed-layout kernel):
  * Host sorts rows by label; positives collapse into a narrow block-diagonal
    band (W columns per core) -> phase 2 (ln + hinge) shrinks ~16x.
  * NORMAL layout (i on partitions): row sums ride the Exp activation's
    accumulator (no more one-hot reduction matmuls), ns = total - samesum
    with samesum a one-DVE masked sum over the band.
  * fp8 e4m3 DoubleRow GEMM (2 matmuls per 512-K psum tile instead of 4);
    ||b||^2 rides the Sqrt activation bias (fp32 exact), ||a||^2 rides a
    K=2 bf16 hi/lo augmented matmul.
  * Per-core 128-aligned column ROTATION (host-side) makes every band slice
    core-uniform so one SPMD program serves all 8 cores; the gathered ns is
    re-indexed on-device via a host-built one-hot selector (eqx) + ones
    matmul instead of a core-dependent DMA offset.
  * The diagonal (i==j) is excluded by the hinge mask directly; no separate
    diagonal correction path at all.
"""

import re
import operator
import numpy as np
import ml_dtypes
from contextlib import ExitStack

import concourse.bass as bass
import concourse.tile as tile
from concourse import bacc, mybir
from concourse import dve_ops
from concourse.dve_spec import Spec, Src0, Src1, C0, relu, sq
from concourse.bass_utils import run_bass_kernel_spmd
from concourse.tile_rust import add_dep_helper

F32 = mybir.dt.float32
BF16 = mybir.dt.bfloat16
FP8 = mybir.dt.float8e4
NPBF16 = ml_dtypes.bfloat16
NPE4M3 = ml_dtypes.float8_e4m3
AF = mybir.ActivationFunctionType
ALU = mybir.AluOpType
DR = mybir.MatmulPerfMode.DoubleRow

N = 4096          # rows (a and b)
F = 512           # features
NCORES = 8
R = N // NCORES   # rows of b per core = 512
TI = R // 128     # i-tiles per core = 4
NJ = N // 512     # j-chunks of 512 = 8
NCLS = 16


def _register_custom(name, spec):
    for op in dve_ops.OPS:
        if op.name == name:
            return op
    op = dve_ops.DveOp(name, spec, subdim=False, uops_sha={})
    dve_ops._SUB_OPCODE_FOR_NAME[name] = (
        max(dve_ops._SUB_OPCODE_FOR_NAME.values()) + 1)
    assert dve_ops._SUB_OPCODE_FOR_NAME[name] < 0x20
    for ver in ("v3", "v4"):
        try:
            op.compile(ver)
        except ValueError as e:
            m = re.search(r"\(%s: ([0-9a-f]+) " % ver, str(e))
            if not m:
                raise
            op.uops_sha[ver] = m.group(1)
            op.compile(ver)
    dve_ops.OPS.append(op)
    dve_ops.CUSTOM_DVE_SPECS[name] = op.spec
    return op


def _register_sqrelu_add():
    """Custom fused DVE op: out = relu(in0 + in1)^2, accum_out = c0 + sum(out)."""
    def _ref(in0, in1, c0, c1, c2):
        out = np.square(np.maximum(
            in0.astype(np.float32) + in1.astype(np.float32), 0.0))
        acc = c0 + out.reshape(out.shape[0], -1).sum(axis=1, keepdims=True)
        return out, acc

    return _register_custom(
        "SQRELU_ADD_ANT",
        Spec(body=sq(relu(Src0 + Src1)), accum=operator.add, accum_init=C0,
             reference=_ref))


def _register_mul_acc():
    """Custom fused DVE op: out = in0 * in1, accum_out = c0 + sum(out)."""
    def _ref(in0, in1, c0, c1, c2):
        out = in0.astype(np.float32) * in1.astype(np.float32)
        acc = c0 + out.reshape(out.shape[0], -1).sum(axis=1, keepdims=True)
        return out, acc

    return _register_custom(
        "MUL_ACC_ANT",
        Spec(body=Src0 * Src1, accum=operator.add, accum_init=C0,
             reference=_ref))


def build_bass(W: int):
    NW = W // 512     # 512-wide band chunks
    NBQ = W // 128
    sqrelu_add = _register_sqrelu_add()
    mul_acc = _register_mul_acc()

    nc = bacc.Bacc("TRN2", target_bir_lowering=False, debug=False,
                   num_devices=NCORES)

    # ---- kernel I/O (per-core shards prepared on host) ----
    at8 = nc.dram_tensor("at8", [128, 4, N], FP8, kind="ExternalInput").ap()      # a_rot.T  [p,k,j]
    bt8 = nc.dram_tensor("bt8", [128, 4, R], FP8, kind="ExternalInput").ap()      # (-2 b_c).T [p,k,i]
    aabc = nc.dram_tensor("aabc", [128, NJ, 512], F32, kind="ExternalInput").ap() # ||a_j||^2 bcast (rotated)
    bb4 = nc.dram_tensor("bb4", [128, TI], F32, kind="ExternalInput").ap()        # ||b_i||^2
    msame = nc.dram_tensor("msame", [128, TI, NW, 512], BF16, kind="ExternalInput").ap()
    mneg = nc.dram_tensor("mneg", [128, TI, NW, 512], BF16, kind="ExternalInput").ap()
    eqx = nc.dram_tensor("eqx", [32, W], BF16, kind="ExternalInput").ap()         # gather-block select
    eye128 = nc.dram_tensor("eye128", [128, 128], F32, kind="ExternalInput").ap()

    out_same = nc.dram_tensor("out_same", [1, 1], F32, kind="ExternalOutput").ap()
    out_ns = nc.dram_tensor("out_ns", [1, R], F32, kind="ExternalOutput").ap()

    with tile.TileContext(nc) as tc, ExitStack() as ctx:
        sb = ctx.enter_context(tc.tile_pool(name="sb", bufs=1))
        dexp_p = ctx.enter_context(tc.tile_pool(name="dexp", bufs=2))
        lp = ctx.enter_context(tc.tile_pool(name="lp", bufs=2))
        scr = ctx.enter_context(tc.tile_pool(name="scr", bufs=2))
        dram = ctx.enter_context(tc.tile_pool(name="dram", bufs=1, space="DRAM"))

        # ---- collective warmup first: fire w1 at t~0 on garbage data (its
        # output is discarded; only the ~50us ncfw init matters) ----
        cc_in = dram.tile([1, R], F32)
        cc_out = dram.tile([1, N], F32)
        warm_in = dram.tile([1, 8], F32)
        warm_out = dram.tile([1, 8 * NCORES], F32)
        with tc.high_priority():
            w1 = nc.gpsimd.collective_compute(
                "AllGather", ALU.bypass,
                replica_groups=[list(range(NCORES))],
                ins=[warm_in[:].opt()], outs=[warm_out[:].opt()])

        # ---- PE pre-warm: dummy matmuls so HAM reaches 8/8 before the real
        # GEMM on EVERY core (cold cores straggle the collective) ----
        wsrc = sb.tile([128, 512], BF16)
        with tc.high_priority():
            nc.vector.memset(wsrc, 0.0)
            with tc.tile_pool(name="warm_ps", bufs=1, space="PSUM") as wpool:
                wps = wpool.tile([128, 512], F32)
                for _ in range(10):
                    nc.tensor.matmul(out=wps, lhsT=wsrc[:, 0:128], rhs=wsrc,
                                     start=True, stop=True)

        # ---- resident SBUF tensors (GEMM-critical first) ----
        bt8_sb = sb.tile([128, 4, R], FP8)
        nc.sync.dma_start(out=bt8_sb, in_=bt8)
        bb_sb = sb.tile([128, TI], F32)
        nc.sync.dma_start(out=bb_sb, in_=bb4)
        at8_sb = sb.tile([128, 4, N], FP8)
        aabc_sb = sb.tile([128, NJ, 512], F32)
        for q in range(8):
            nc.sync.dma_start(out=at8_sb[:, :, 512 * q:512 * q + 512],
                              in_=at8[:, :, 512 * q:512 * q + 512])
            nc.sync.dma_start(out=aabc_sb[:, q, :], in_=aabc[:, q, :])

        ones32 = sb.tile([32, 128], BF16)
        nc.vector.memset(ones32, 1.0)
        ones128c = sb.tile([128, 1], F32)
        nc.vector.memset(ones128c, 1.0)

        dT = sb.tile([128, TI, NJ, 512], F32)     # D, 64KB/partition
        dm_sb = sb.tile([128, TI, NW, 512], F32)  # D + hinge mask over band
        tot_sb = sb.tile([128, TI], F32)
        sam_sb = sb.tile([128, TI], F32)
        ns_sb = sb.tile([128, TI], F32)
        hacc_sb = sb.tile([128, TI], F32)
        nst_sb = sb.tile([TI, 128], F32)
        nsflat_sb = sb.tile([32, 128], F32)
        nsqe_sb = sb.tile([32, W], BF16)

        # later resident loads (not on the GEMM critical path)
        msame_sb = sb.tile([128, TI, NW, 512], BF16)
        nc.sync.dma_start(out=msame_sb, in_=msame)
        mneg_sb = sb.tile([128, TI, NW, 512], BF16)
        nc.sync.dma_start(out=mneg_sb, in_=mneg)
        eqx_sb = sb.tile([32, W], BF16)
        nc.sync.dma_start(out=eqx_sb, in_=eqx)
        eye128_sb = sb.tile([128, 128], F32)
        nc.sync.dma_start(out=eye128_sb, in_=eye128)

        # ========== PHASE 1: GEMM (fp8 DoubleRow) -> +aa (DVE) -> sqrt =====
        sqrt_insts = []
        with tc.tile_pool(name="dsq_ps", bufs=2, space="PSUM") as dsq_pool, \
             tc.tile_pool(name="dsq_sb", bufs=2) as dsq_sbp:
            for t in range(TI):
                lw01 = bt8_sb[:, 0:2, 128 * t:128 * t + 128]
                lw23 = bt8_sb[:, 2:4, 128 * t:128 * t + 128]
                for h in range(2):
                    ps = dsq_pool.tile([128, 4, 512], F32, tag="dsq")
                    for u in range(4):
                        q = 4 * h + u
                        jsl = slice(512 * q, 512 * q + 512)
                        nc.tensor.matmul(out=ps[:, u, :], lhsT=lw01,
                                         rhs=at8_sb[:, 0:2, jsl],
                                         perf_mode=DR, start=True, stop=False)
                        nc.tensor.matmul(out=ps[:, u, :], lhsT=lw23,
                                         rhs=at8_sb[:, 2:4, jsl],
                                         perf_mode=DR, start=False, stop=True)
                    dsq = dsq_sbp.tile([128, 4, 512], F32, tag="dsqs")
                    nc.vector.tensor_add(out=dsq, in0=ps,
                                         in1=aabc_sb[:, 4 * h:4 * h + 4, :])
                    si = nc.scalar.activation(
                        out=dT[:, t, 4 * h:4 * h + 4, :], in_=dsq,
                        func=AF.Sqrt, bias=bb_sb[:, t:t + 1], scale=1.0)
                    sqrt_insts.append(si)

        # ========== exp (+row totals via ACT accumulate), samesum ==========
        prev = sqrt_insts[-1]
        for t in range(TI):
            dexp = dexp_p.tile([128, NJ, 512], BF16, tag="dexp")
            ei = nc.scalar.activation(
                out=dexp, in_=dT[:, t, :, :], func=AF.Exp,
                scale=-1.0, bias=1.0, accum_out=tot_sb[:, t:t + 1])
            add_dep_helper(ei.ins, prev.ins, False, "ACT table order")
            prev = ei
            sscr = scr.tile([128, NW, 512], BF16, tag="sscr")
            nc.vector._custom_dve(mul_acc, out=sscr, in0=dexp[:, 0:NW, :],
                                  in1=msame_sb[:, t, :, :],
                                  s0=0.0, accum_out=sam_sb[:, t:t + 1])
            # hinge-masked D band (runs on DVE during the exp window)
            nc.vector.tensor_add(out=dm_sb[:, t, :, :],
                                 in0=dT[:, t, 0:NW, :],
                                 in1=mneg_sb[:, t, :, :])

        nc.vector.tensor_sub(ns_sb, tot_sb, sam_sb)

        # ========== ns -> row layout -> AllGather ==========
        with tc.tile_pool(name="ps2", bufs=1, space="PSUM") as ps2, \
             tc.tile_pool(name="nsbc_ps", bufs=1, space="PSUM") as nsbc_pool:
            nst_ps = ps2.tile([TI, 128], F32, tag="nst")
            nc.tensor.matmul(out=nst_ps, lhsT=ns_sb, rhs=eye128_sb,
                             start=True, stop=True)
            nc.vector.tensor_copy(out=nst_sb, in_=nst_ps)
            nc.sync.dma_start(out=cc_in[0, :].rearrange("(t p) -> t p", p=128),
                              in_=nst_sb)
            nc.sync.dma_start(out=out_ns[0, :].rearrange("(t p) -> t p", p=128),
                              in_=nst_sb)
            cc_inst = nc.gpsimd.collective_compute(
                "AllGather", ALU.bypass,
                replica_groups=[list(range(NCORES))],
                ins=[cc_in[:].opt()], outs=[cc_out[:].opt()])
            rd = nc.sync.dma_start(
                out=nsflat_sb, in_=cc_out[0, :].rearrange("(q p) -> q p", p=128))
            add_dep_helper(rd.ins, cc_inst.ins, True, "read ns after collective")

            # ns_bc[p, x] = ns[(s_c + x) mod N]  (selector matmul, core-uniform)
            nc.vector.tensor_mul(
                out=nsqe_sb.rearrange("q (y m) -> q y m", m=128),
                in0=nsflat_sb.rearrange("q (o p) -> q o p", o=1)
                    .broadcast_to([32, NBQ, 128]),
                in1=eqx_sb.rearrange("q (y m) -> q y m", m=128))
            nsbc_ps = nsbc_pool.tile([128, NW, 512], F32, tag="nsbc")
            for v in range(NW):
                nc.tensor.matmul(out=nsbc_ps[:, v, :], lhsT=ones32,
                                 rhs=nsqe_sb[:, 512 * v:512 * v + 512],
                                 start=True, stop=True)

            # ========== PHASE 2: J = ln(ns_i + ns_j) + D; hinge^2 ==========
            for t in range(TI):
                L = lp.tile([128, NW, 512], F32, tag="L")
                nc.scalar.activation(out=L, in_=nsbc_ps, func=AF.Ln,
                                     bias=ns_sb[:, t:t + 1], scale=1.0)
                h2 = scr.tile([128, NW, 512], BF16, tag="h2")
                nc.vector._custom_dve(sqrelu_add, out=h2, in0=L,
                                      in1=dm_sb[:, t, :, :],
                                      s0=0.0, accum_out=hacc_sb[:, t:t + 1])

            fin_ps = ps2.tile([1, TI], F32, tag="fin")
            nc.tensor.matmul(out=fin_ps, lhsT=ones128c, rhs=hacc_sb,
                             start=True, stop=True)
            same_sum = sb.tile([1, 1], F32)
            nc.vector.reduce_sum(out=same_sum, in_=fin_ps,
                                 axis=mybir.AxisListType.X)
            nc.sync.dma_start(out=out_same, in_=same_sum)

    nc.compile()
    return nc


_CACHE: dict = {}


def _get_nc(W: int):
    key = ("nc", W)
    if key not in _CACHE:
        _CACHE[key] = build_bass(W)
    return _CACHE[key]


def prepare_inputs(a: np.ndarray, b: np.ndarray, labels: np.ndarray):
    """Host-side sharding/layout prep. Returns (per-core input maps, num_pos, W)."""
    a = np.asarray(a, np.float32)
    b = np.asarray(b, np.float32)
    labels = np.asarray(labels).astype(np.int64)
    perm = np.argsort(labels, kind="stable")
    a_s, b_s, ls = a[perm], b[perm], labels[perm]
    counts = np.bincount(ls, minlength=NCLS)
    starts = np.concatenate([[0], np.cumsum(counts)])
    num_pos = float((counts.astype(np.float64) ** 2).sum() - N)

    bands = []
    for c in range(NCORES):
        l0, l1 = ls[R * c], ls[R * c + R - 1]
        bands.append((int(starts[l0]), int(starts[l1 + 1])))
    maxw = max(j1 - 128 * (j0 // 128) for j0, j1 in bands)
    W = int(np.ceil(maxw / 512) * 512)

    aa = np.sum(a_s * a_s, axis=1, dtype=np.float32)
    bb = np.sum(b_s * b_s, axis=1, dtype=np.float32)
    eye = np.eye(128, dtype=np.float32)

    in_maps = []
    for c in range(NCORES):
        j0, _ = bands[c]
        s = 128 * (j0 // 128)
        cols = (s + np.arange(N)) % N
        a_rot = a_s[cols]
        at8 = np.ascontiguousarray(
            a_rot.T.reshape(4, 128, N).transpose(1, 0, 2)).astype(NPE4M3)
        aar = aa[cols]
        aabc = np.ascontiguousarray(
            np.broadcast_to(aar[None, :], (128, N)).reshape(128, NJ, 512))
        sl = slice(R * c, R * c + R)
        bt8 = np.ascontiguousarray(
            (-2.0 * b_s[sl]).T.reshape(4, 128, R).transpose(1, 0, 2)).astype(NPE4M3)
        bb4 = np.ascontiguousarray(bb[sl].reshape(TI, 128).T)

        collab = ls[cols[:W]]
        rowlab = np.ascontiguousarray(ls[sl].reshape(TI, 128).T)
        same = rowlab[:, :, None] == collab[None, None, :]      # [128, TI, W]
        gidx = (R * c + 128 * np.arange(TI)[None, :]
                + np.arange(128)[:, None])
        isdiag = gidx[:, :, None] == cols[:W][None, None, :]
        msame = same.astype(NPBF16).reshape(128, TI, W // 512, 512)
        mneg = np.where(same & ~isdiag, np.float32(0.0),
                        np.float32(-30000.0)).astype(NPBF16)
        mneg = mneg.reshape(128, TI, W // 512, 512)
        qb0 = s // 128
        eqxm = (np.arange(32)[:, None]
                == ((qb0 + np.arange(W) // 128) % 32)[None, :]).astype(NPBF16)
        in_maps.append({
            "at8": at8, "bt8": bt8, "aabc": aabc, "bb4": bb4,
            "msame": np.ascontiguousarray(msame),
            "mneg": np.ascontiguousarray(mneg),
            "eqx": np.ascontiguousarray(eqxm), "eye128": eye,
        })
    return in_maps, num_pos, W


def run(a, b, labels, trace=False, trace_kwargs=None):
    """Run on 8 NeuronCores; returns (loss, BassKernelResults)."""
    in_maps, num_pos, W = prepare_inputs(a, b, labels)
    nc = _get_nc(W)
    kw = {}
    if trace:
        kw = dict(trace=True, **(trace_kwargs or {}))
    res = run_bass_kernel_spmd(nc, in_maps, core_ids=list(range(NCORES)), **kw)

    total = 0.0
    for c in range(NCORES):
        total += float(res.results[c]["out_same"][0, 0])
    loss = total / (2.0 * num_pos)
    return np.asarray(np.float32(loss)), res


def kernel(a, b, labels):
    loss, _ = run(a, b, labels)
    return loss
